# revision 2
# baseline (speedup 1.0000x reference)
"""Trainium2 Bass kernel v2 for nn_CCSequenceModel.

Layout: pure data parallel over 8 cores (2048 batch each), batch folded:
half A (1024) on partitions 0-63, half B on partitions 64-127; free dim
NW=1024 processed in 2 chunks of 512 (one PSUM bank per gate-chunk).

v2 vs baseline:
- Block-diagonal duplicated weights: each gate contraction is ONE
  128x128-array matmul covering both batch halves (was 2 quadrant MMs),
  halving MM/LDWEIGHTS instruction count and making full-array use
  unconditional.
- ScalarE trimmed to sigmoid_r, sigmoid_z, tanh per chunk (bias of the
  n-gate folded into the tanh activation bias; Identity op removed).
- Sm = inn + r*(hn+bhh_n) split: the psum-side add runs on GPSIMD (idle
  otherwise), freeing DVE for the h' update chain.
- Decoder heads packed: logit/cv for all (chunk, half) combos land on
  psum partitions 0-3 of two tiles -> one is_gt + one gated-multiply DVE
  op per STEP (was 4 full-width ops), and the gated output tile PVpack
  [4, 512] directly feeds the next step's d0 input contraction (K=4
  matmuls with one-hot weight rows).
"""

import os
import sys

sys.path.insert(0, "/opt/trn_rl_repo")

import numpy as np
import ml_dtypes

import concourse.bass as bass
import concourse.bacc as bacc_mod
import concourse.mybir as mybir
import concourse.tile as tile
from concourse.bass_utils import run_bass_kernel_spmd

F32 = mybir.dt.float32
BF16 = mybir.dt.bfloat16
AF = mybir.ActivationFunctionType
OP = mybir.AluOpType

H = 64
NCORES = 8
BLOC = 2048          # batch per core
NW = 1024            # folded free width (batch half)
CH = 512             # chunk of NW (one psum bank of fp32)
BF16_T = ml_dtypes.bfloat16

CELLS = ["e0", "e1", "d0", "d1"]


def _wcols():
    """wpack column map. Blocks:
    - '<cell>hh_<g>'  [128,128] block-diag dup of Whh_g.T
    - '<cell>ih_<g>'  e1/d1: [128,128] block-diag dup of Wih_g.T
                      e0:    [*,64]  Wih_g.T at rows 0-5 and 64-69
                      d0:    [4,64]x4 one-hot-row variants (cols 64/variant)
    - 'on<c>'/'cv<c>' heads: [128,4]: rows 0-63 col 2c+0 = w, rows 64-127
                      col 2c+1 = w  (c = chunk)
    """
    cols = {}
    cur = 0
    for c in CELLS:
        for g in ("r", "z", "n"):
            cols[c + "hh_" + g] = cur
            cur += 128
    for c in ("e1", "d1"):
        for g in ("r", "z", "n"):
            cols[c + "ih_" + g] = cur
            cur += 128
    for g in ("r", "z", "n"):
        cols["e0ih_" + g] = cur
        cur += 128
    for g in ("r", "z", "n"):
        for v in range(2):
            cols[f"d0ih_{g}{v}"] = cur
            cur += 128
    for c in range(2):
        cols[f"on{c}"] = cur
        cur += 4
        cols[f"cv{c}"] = cur
        cur += 4
    cols["idm"] = cur
    cur += 128
    return cols, cur


def _bcols():
    cols = {}
    cur = 0
    for c in CELLS:
        for p in ["sr", "sz", "msz", "bin", "bhn"]:
            cols[c + p] = cur
            cur += 1
    cols["bon4"] = cur
    cur += 1
    cols["bcv4"] = cur
    cur += 1
    return cols, cur


WCOLS, NWCOL = _wcols()
BCOLS, NBCOL = _bcols()


def pack_weights(iv):
    wp = np.zeros((128, NWCOL), np.float32)
    bp = np.zeros((128, NBCOL), np.float32)

    def put_diag(col, wt):
        # wt [K, 64] = W.T ; block-diag dup into [128, 128] at col
        k = wt.shape[0]
        wp[0:k, col : col + 64] = wt
        wp[64 : 64 + k, col + 64 : col + 128] = wt

    def put_b(col, v):
        v = np.asarray(v, np.float32).reshape(-1)
        bp[0 : v.size, col] = v
        bp[64 : 64 + v.size, col] = v

    cfg = {
        "e0": ("e_Wih0", "e_Whh0", "e_bih0", "e_bhh0"),
        "e1": ("e_Wih1", "e_Whh1", "e_bih1", "e_bhh1"),
        "d0": ("d_Wih0", "d_Whh0", "d_bih0", "d_bhh0"),
        "d1": ("d_Wih1", "d_Whh1", "d_bih1", "d_bhh1"),
    }
    for c, (wih, whh, bih, bhh) in cfg.items():
        Wih = np.asarray(iv[wih], np.float32)
        Whh = np.asarray(iv[whh], np.float32)
        bi, bh = np.asarray(iv[bih], np.float32), np.asarray(iv[bhh], np.float32)
        for gi, g in enumerate(["r", "z", "n"]):
            wg_ih = Wih[gi * 64 : (gi + 1) * 64, :].T  # [K, 64]
            wg_hh = Whh[gi * 64 : (gi + 1) * 64, :].T
            put_diag(WCOLS[c + "hh_" + g], wg_hh)
            if c in ("e1", "d1"):
                put_diag(WCOLS[c + "ih_" + g], wg_ih)
            elif c == "e0":
                col = WCOLS["e0ih_" + g]
                wp[0:6, col : col + 64] = wg_ih
                wp[64:70, col + 64 : col + 128] = wg_ih
            else:  # d0: per-chunk block: row 2c -> A cols, 2c+1 -> B cols
                for v in range(2):
                    col = WCOLS[f"d0ih_{g}{v}"]
                    wp[2 * v, col : col + 64] = wg_ih[0]
                    wp[2 * v + 1, col + 64 : col + 128] = wg_ih[0]
        put_b(BCOLS[c + "sr"], bi[0:64] + bh[0:64])
        put_b(BCOLS[c + "sz"], bi[64:128] + bh[64:128])
        put_b(BCOLS[c + "msz"], -(bi[64:128] + bh[64:128]))
        put_b(BCOLS[c + "bin"], bi[128:192])
        put_b(BCOLS[c + "bhn"], bh[128:192])
    won = np.asarray(iv["W_on"], np.float32).reshape(64)
    wcv = np.asarray(iv["W_cv"], np.float32).reshape(64)
    # heads: chunk c -> out part 2c (half A, K rows 0-63) and part
    # 2c+1 (half B, rows 64-127); per-chunk 4-col lhsT variant with the
    # other chunk's columns zeroed (the two MMs accumulate into [4, CH])
    for c in range(2):
        wp[0:64, WCOLS[f"on{c}"] + 2 * c] = won
        wp[64:128, WCOLS[f"on{c}"] + 2 * c + 1] = won
        wp[0:64, WCOLS[f"cv{c}"] + 2 * c] = wcv
        wp[64:128, WCOLS[f"cv{c}"] + 2 * c + 1] = wcv
    wp[:, WCOLS["idm"] : WCOLS["idm"] + 128] = np.eye(128, dtype=np.float32)
    bp[:, BCOLS["bon4"]] = float(np.asarray(iv["b_on"]).reshape(()))
    bp[:, BCOLS["bcv4"]] = float(np.asarray(iv["b_cv"]).reshape(()))
    return wp.astype(BF16_T), bp


def build_program(T, DEC):
    KPRE = 0
    nc = bacc_mod.Bacc(None, target_bir_lowering=False)
    xpack = nc.declare_dram_parameter("xpack", [T, 70, NW], BF16, isOutput=False)
    wpack = nc.declare_dram_parameter("wpack", [128, NWCOL], BF16, isOutput=False)
    bpack = nc.declare_dram_parameter("bpack", [128, NBCOL], F32, isOutput=False)
    outd = nc.declare_dram_parameter("out", [DEC, 4, CH], BF16, isOutput=True)

    with tile.TileContext(nc) as tc:
        with (
            tc.tile_pool(name="const", bufs=1) as const,
            tc.tile_pool(name="state", bufs=1) as state,
            tc.tile_pool(name="xin", bufs=4) as xin,
            tc.tile_pool(name="tmp", bufs=6) as tmp,
            tc.tile_pool(name="pvp", bufs=2) as pvp,
            tc.tile_pool(name="psum", bufs=1, space="PSUM") as psum,
        ):
            wsb = const.tile([128, NWCOL], BF16)
            bsb = const.tile([128, NBCOL], F32)
            nc.gpsimd.dma_start(out=wsb[:, :], in_=wpack[:, :])
            nc.gpsimd.dma_start(out=bsb[:, :], in_=bpack[:, :])

            S0 = state.tile([128, NW], BF16, tag="S0")
            S1 = state.tile([128, NW], BF16, tag="S1")
            PVz = state.tile([128, CH], BF16, tag="PVz")  # zero first prev
            nc.vector.memset(S0[:, :], 0.0)
            nc.vector.memset(S1[:, :], 0.0)
            nc.vector.memset(PVz[:, :], 0.0)

            def W(name, r0, k, m):
                c = WCOLS[name]
                return wsb[r0 : r0 + k, c : c + m]

            def B_(name):
                c = BCOLS[name]
                return bsb[:, c : c + 1]

            def mm(out, lhsT, rhs, start, stop):
                nc.tensor.matmul(out, lhsT, rhs, start=start, stop=stop,
                                 skip_group_check=True)

            # ---- matmul emission helpers (per cell, per chunk) ----
            def emit_ih(cell, c, P, xst=None, pv=None):
                """ih contraction for all 3 gates of chunk c into psum dict P.
                e1/d1: block-diag from S0. e0: quadrant pair from xst.
                d0: K=4 one-hot pair from pv."""
                sl = slice(c * CH, (c + 1) * CH)
                for g, tag in (("r", "P_r"), ("z", "P_z"), ("n", "P_i")):
                    dst = P[tag]
                    last = g != "n"  # r/z groups end with ih (hh came first)
                    if cell in ("e1", "d1"):
                        mm(dst[:, :], W(cell + "ih_" + g, 0, 128, 128),
                           S0[:, sl], start=(g == "n"), stop=True)
                    elif cell == "e0":
                        mm(dst[0:64, :], W("e0ih_" + g, 0, 6, 64),
                           xst[0:6, sl], start=(g == "n"), stop=True)
                        mm(dst[64:128, :], W("e0ih_" + g, 64, 6, 64),
                           xst[64:70, sl], start=(g == "n"), stop=True)
                    else:  # d0
                        mm(dst[0:64, :], W(f"d0ih_{g}{2 * c}", 0, 4, 64),
                           pv[0:4, :], start=(g == "n"), stop=True)
                        mm(dst[64:128, :], W(f"d0ih_{g}{2 * c + 1}", 0, 4, 64),
                           pv[0:4, :], start=(g == "n"), stop=True)

            def emit_hh(cell, c, P, hS):
                sl = slice(c * CH, (c + 1) * CH)
                for g, tag in (("r", "P_r"), ("z", "P_z"), ("n", "P_h")):
                    dst = P[tag]
                    mm(dst[:, :], W(cell + "hh_" + g, 0, 128, 128),
                       hS[:, sl], start=True, stop=(g == "n"))

            def alloc_P(layer, c):
                # r/z per-layer; n-side shared across layers (disjoint
                # lifetimes); heads use the two remaining banks
                def tg(k):
                    return k + (f"{layer}" if k in ("P_r", "P_z") else "")
                return {
                    k: psum.tile([128, CH], F32, tag=tg(k), name=f"{k}{layer}_{c}")
                    for k in ("P_r", "P_z", "P_i", "P_h")
                }

            def elementwise(cell, c, P, outS):
                """sigmoids, n-assembly, h' update for chunk c."""
                sl = slice(c * CH, (c + 1) * CH)
                R = tmp.tile([128, CH], BF16, tag="R")
                Z = tmp.tile([128, CH], BF16, tag="Z")
                A_ = tmp.tile([128, CH], BF16, tag="A")
                Sm = tmp.tile([128, CH], BF16, tag="Sm")
                N_ = tmp.tile([128, CH], BF16, tag="N")
                D = tmp.tile([128, CH], BF16, tag="D")
                E = tmp.tile([128, CH], BF16, tag="E")
                nc.scalar.activation(R[:, :], P["P_r"][:, :], AF.Sigmoid,
                                     bias=B_(cell + "sr"))
                # A = (hn + bhh_n) * r
                nc.vector.scalar_tensor_tensor(
                    out=A_[:, :], in0=P["P_h"][:, :], scalar=B_(cell + "bhn"),
                    in1=R[:, :], op0=OP.add, op1=OP.mult)
                nc.scalar.activation(Z[:, :], P["P_z"][:, :], AF.Sigmoid,
                                     bias=B_(cell + "sz"))
                # Sm = inn + A   (GPSIMD; bih_n folded into tanh bias)
                nc.gpsimd.tensor_tensor(out=Sm[:, :], in0=P["P_i"][:, :],
                                        in1=A_[:, :], op=OP.add)
                nc.scalar.activation(N_[:, :], Sm[:, :], AF.Tanh,
                                     bias=B_(cell + "bin"))
                # h' = n + z*(h - n)
                nc.vector.tensor_tensor(out=D[:, :], in0=outS[:, sl],
                                        in1=N_[:, :], op=OP.subtract)
                nc.vector.tensor_tensor(out=E[:, :], in0=Z[:, :], in1=D[:, :],
                                        op=OP.mult)
                nc.vector.tensor_tensor(out=outS[:, sl], in0=N_[:, :],
                                        in1=E[:, :], op=OP.add)

            # ---------------- encoder ----------------
            for t in range(T):
                xst = xin.tile([70, NW], BF16, tag="xst")
                nc.gpsimd.dma_start(out=xst[0:6, :], in_=xpack[t, 0])
                nc.gpsimd.dma_start(out=xst[64:70, :], in_=xpack[t, 1])
                P0 = [alloc_P(0, c) for c in range(2)]
                P1 = [alloc_P(1, c) for c in range(2)]
                # e0: hh first (S0 old), then ih; per chunk
                for c in range(2):
                    emit_hh("e0", c, P0[c], S0)
                    emit_ih("e0", c, P0[c], xst=xst)
                # e1 hh: depends only on old S1 -> ready now
                for c in range(2):
                    emit_hh("e1", c, P1[c], S1)
                for c in range(2):
                    elementwise("e0", c, P0[c], S0)
                # e1 ih: needs new S0 chunk
                for c in range(2):
                    emit_ih("e1", c, P1[c])
                for c in range(2):
                    elementwise("e1", c, P1[c], S1)

            # ---------------- decoder ----------------
            PV = PVz
            for t in range(DEC):
                P0 = [alloc_P(0, c) for c in range(2)]
                P1 = [alloc_P(1, c) for c in range(2)]
                for c in range(2):
                    emit_hh("d0", c, P0[c], S0)
                    emit_ih("d0", c, P0[c], pv=PV)
                for c in range(2):
                    emit_hh("d1", c, P1[c], S1)
                for c in range(2):
                    elementwise("d0", c, P0[c], S0)
                for c in range(2):
                    emit_ih("d1", c, P1[c])
                for c in range(2):
                    elementwise("d1", c, P1[c], S1)
                # heads: logit/cv of new S1 -> PVpack [4, CH]
                P_on = psum.tile([128, CH], F32, tag="P_r0")
                P_cv = psum.tile([128, CH], F32, tag="P_z0")
                for c in range(2):
                    sl = slice(c * CH, (c + 1) * CH)
                    mm(P_on[0:4, :], W(f"on{c}", 0, 64, 4), S1[0:64, sl],
                       start=(c == 0), stop=False)
                    mm(P_on[0:4, :], W(f"on{c}", 64, 64, 4), S1[64:128, sl],
                       start=False, stop=(c == 1))
                    mm(P_cv[0:4, :], W(f"cv{c}", 0, 64, 4), S1[0:64, sl],
                       start=(c == 0), stop=False)
                    mm(P_cv[0:4, :], W(f"cv{c}", 64, 64, 4), S1[64:128, sl],
                       start=False, stop=(c == 1))
                MK = tmp.tile([128, CH], BF16, tag="MK")
                PVn = pvp.tile([128, CH], BF16, tag="PV")
                # mask = (logit + b_on) > 0
                nc.vector.tensor_scalar(
                    out=MK[0:4, :], in0=P_on[0:4, :],
                    scalar1=bsb[0:4, BCOLS["bon4"] : BCOLS["bon4"] + 1],
                    scalar2=0.0, op0=OP.add, op1=OP.is_gt)
                # gated = (cv + b_cv) * mask
                nc.vector.scalar_tensor_tensor(
                    out=PVn[0:4, :], in0=P_cv[0:4, :],
                    scalar=bsb[0:4, BCOLS["bcv4"] : BCOLS["bcv4"] + 1],
                    in1=MK[0:4, :], op0=OP.add, op1=OP.mult)
                nc.sync.dma_start(out=outd[t], in_=PVn[0:4, :])
                PV = PVn
    nc.compile()
    return nc


_CACHE = {}


def get_program(T, DEC):
    key = (T, DEC)
    if key not in _CACHE:
        _CACHE[key] = build_program(T, DEC)
    return _CACHE[key]


def pack_x(x):
    """x [B, T, NI] f32 -> per-core xpack [T, 70, NW] bf16 (zero-padded
    rows 6-63 so e0's ih contraction is one block-diag matmul)."""
    B, T, NI = x.shape
    out = []
    for c in range(NCORES):
        xs = x[c * BLOC : (c + 1) * BLOC]
        xp = xs.reshape(2, NW, T, NI).transpose(2, 0, 3, 1)  # [T, 2, 6, NW]
        full = np.zeros((T, 70, NW), np.float32)
        full[:, 0:6] = xp[:, 0]
        full[:, 64:70] = xp[:, 1]
        out.append(full.astype(BF16_T))
    return out


def run(x, target_len, weights, trace=False, trace_kwargs=None):
    T = x.shape[1]
    DEC = int(target_len)
    nc = get_program(T, DEC)
    wp, bp = pack_weights(weights)
    xps = pack_x(np.asarray(x, np.float32))
    in_maps = [{"xpack": xps[c], "wpack": wp, "bpack": bp} for c in range(NCORES)]
    res = run_bass_kernel_spmd(
        nc, in_maps, list(range(NCORES)), trace=trace, **(trace_kwargs or {})
    )
    outs = [np.asarray(res.results[c]["out"], np.float32) for c in range(NCORES)]
    # [DEC, 4, CH] parts (c0A, c0B, c1A, c1B) -> [BLOC, DEC, 1]
    full = np.zeros((NCORES * BLOC, DEC, 1), np.float32)
    for ci, o in enumerate(outs):
        b0 = ci * BLOC
        full[b0 + 0 : b0 + 512, :, 0] = o[:, 0, :].T
        full[b0 + 1024 : b0 + 1536, :, 0] = o[:, 1, :].T
        full[b0 + 512 : b0 + 1024, :, 0] = o[:, 2, :].T
        full[b0 + 1536 : b0 + 2048, :, 0] = o[:, 3, :].T
    return full, res


def kernel(**inputs):
    x = np.asarray(inputs["x"], np.float32)
    target_len = int(np.asarray(inputs["target_len"]).reshape(()))
    weights = {k: v for k, v in inputs.items() if k not in ("x", "target_len")}
    full, _ = run(x, target_len, weights)
    return full.astype(np.float32)
